# revision 3
# baseline (speedup 1.0000x reference)
"""Trainium2 Bass kernel for nn_DeepLinear (784->10 linear + BN, 62x(10->10 linear + BN), 10->10 linear).

Math: training-mode BN fixes each layer's output batch mean (beta) and variance
(gamma^2), so every layer past the first acts *linearly* on the centered
activations of h = x @ W0.T. The whole net collapses to:
    h  = x @ W0.T                      (heavy; on device, data-parallel over batch)
    mu = mean(h), C = cov(h)           (global batch moments; host, float64)
    T, r = 62-layer chain of 10x10 covariance algebra   (tiny; host, float64)
    out = h @ T + r                    (light; on device)

Stage 1 streams x as fp8e3m4 (1 byte/elem; ~1.8% rms quantization noise per
element -> ~1.4e-2 relative output error, inside the 2e-2 gate) and runs the
matmul with each x block as the PE *stationary* operand ([112,128]) and the
fp16 weights as the 10-wide moving operand, so a 128-row batch block costs ~10
PE rows and the result lands directly in [batch, feature] layout. h returns as
fp16; the host computes the batch moments from those same fp16 values (the
sync-BN all-reduce), collapses the BN chain, and launches stage 2, which
applies T to 11 batch blocks per matmul via a block-diagonal layout (10
features x 11 blocks on 110 partitions). The bias r is added on host.

Both launches are raw bass (no TileContext): hand-rolled semaphores, per-chunk
DMA-completion sems (DMA engines complete out of order), x-chunk loads on SP
with weight loads and the mid output write on ACT so the serialized DMA stream
stays gap-free, and a Pool epilogue that re-zeros semaphores so repeat
executions of the same NEFF stay correct. Stage 2 burns stage 1's semaphore ID
range so stale (deliberately unwaited) output sems can never alias.
"""

import numpy as np

EPS = 1e-5
B = 65536
D = 784
NCORES = 8
BC = B // NCORES          # 8192 rows per core
KP = 112                  # contraction partitions per chunk (7 * 112 = 784)
KC = 7                    # contraction chunks
BLK = 128                 # batch rows per matmul block
NBLK = BC // BLK          # 64 blocks per core

CHUNKS = [512] * 15 + [384, 128]            # batch cols per x DMA chunk
GROUPS = [(0, 4), (4, 4), (8, 4), (12, 4), (16, 1)]   # (first chunk, n chunks)
G2 = [11, 11, 11, 11, 11, 9]                # stage-2 blocks per matmul

_cache = {}


def _build_stage1():
    import concourse.bacc as bacc
    import concourse.mybir as mybir

    F16 = mybir.dt.float16
    F32 = mybir.dt.float32
    E3 = mybir.dt.float8e3

    nc = bacc.Bacc("TRN2", target_bir_lowering=False, debug=False, num_devices=NCORES)
    xp = nc.dram_tensor("xp", [KC * KP * BC], E3, kind="ExternalInput")
    wm = nc.dram_tensor("wm", [KP, KC * 10], F16, kind="ExternalInput")
    hb = nc.dram_tensor("hb", [128, NBLK * 10], F16, kind="ExternalOutput")

    coff = np.concatenate([[0], np.cumsum(CHUNKS)])
    NCH = len(CHUNKS)
    XBUF = 4

    wm_sb = nc.alloc_sbuf_tensor("wm_sb", [KP, KC * 10], F16)
    hn_sb = nc.alloc_sbuf_tensor("hn_sb", [128, NBLK * 10], F16)
    xts = [nc.alloc_sbuf_tensor(f"xt{i}", [KP, KC, 512], E3) for i in range(XBUF)]
    xt_tail = {
        W: nc.alloc_sbuf_tensor(f"xt_{W}", [KP, KC, W], E3)
        for W in sorted(set(CHUNKS) - {512})
    }
    pss = [
        nc.alloc_psum_tensor(
            f"ps{g}", [128, (coff[c0 + n] - coff[c0]) // BLK * 10], F32
        )
        for g, (c0, n) in enumerate(GROUPS)
    ]

    s_w = nc.alloc_semaphore("s_w")                              # wm loaded
    s_xc = [nc.alloc_semaphore(f"s_x{c}") for c in range(NCH)]   # per-chunk
    s_rd = nc.alloc_semaphore("s_rd")    # chunks fully consumed by PE
    s_cp = nc.alloc_semaphore("s_cp")    # group copies done
    s_ob = nc.alloc_semaphore("s_ob")    # output writes (unwaited)

    nc.scalar.dma_start(wm_sb[:], wm[:]).then_inc(s_w, 16)

    # all x-chunk loads on SP; ring of XBUF buffers for the 512-col chunks
    ring = 0
    for c in range(NCH):
        W = CHUNKS[c]
        if W == 512:
            xt = xts[ring % XBUF]
            if ring >= XBUF:
                nc.sync.wait_ge(s_rd, ring - XBUF + 1)
            ring += 1
        else:
            xt = xt_tail[W]
        pos = KP * KC * coff[c]
        nc.sync.dma_start(
            xt[:, :, 0:W],
            xp[pos:pos + KP * KC * W].rearrange("(p k w) -> p k w", p=KP, k=KC),
        ).then_inc(s_xc[c], 16)

    nc.tensor.wait_ge(s_w, 16)
    ring = 0
    for g, (c0, nch) in enumerate(GROUPS):
        b0 = coff[c0] // BLK
        ps = pss[g]
        for c in range(c0, c0 + nch):
            W = CHUNKS[c]
            if W == 512:
                xt = xts[ring % XBUF]
                ring += 1
            else:
                xt = xt_tail[W]
            nc.tensor.wait_ge(s_xc[c], 16)
            mm = None
            for b in range(W // BLK):
                blk = (coff[c] // BLK - b0) + b
                for k in range(KC):
                    mm = nc.tensor.matmul(
                        ps[:, blk * 10:(blk + 1) * 10],
                        xt[:, k, b * BLK:(b + 1) * BLK],
                        wm_sb[:, k * 10:(k + 1) * 10],
                        start=(k == 0),
                        stop=(k == KC - 1),
                    )
            mm.then_inc(s_rd)
        nb = (coff[c0 + nch] - coff[c0]) // BLK
        if g == len(GROUPS) - 1:
            # last (1-block) group copies on ACT so it runs parallel to g3's
            # DVE copy in the tail
            nc.scalar.wait_ge(s_rd, c0 + nch)
            nc.scalar.activation(
                hn_sb[:, b0 * 10:(b0 + nb) * 10], ps[:],
                mybir.ActivationFunctionType.Copy,
            ).then_inc(s_cp)
        else:
            nc.vector.wait_ge(s_rd, c0 + nch)
            nc.vector.tensor_copy(
                hn_sb[:, b0 * 10:(b0 + nb) * 10], ps[:]
            ).then_inc(s_cp)

    # writes: [0:320] mid-stream on ACT after groups 0-1; rest on SP at the end
    nc.scalar.wait_ge(s_cp, 3)
    nc.scalar.dma_start(hb[:, 0:320], hn_sb[:, 0:320]).then_inc(s_ob, 16)
    nc.sync.wait_ge(s_cp, len(GROUPS))
    nc.sync.dma_start(hb[:, 320:640], hn_sb[:, 320:640]).then_inc(s_ob, 16)

    # epilogue: re-zero semaphores for repeat executions (off critical path)
    nc.gpsimd.wait_ge(s_cp, len(GROUPS))
    for s in (s_w, s_rd, s_cp, *s_xc):
        nc.gpsimd.sem_clear(s)
    nc.finalize()
    return nc


def _build_stage2():
    import concourse.bacc as bacc
    import concourse.mybir as mybir

    F16 = mybir.dt.float16
    F32 = mybir.dt.float32

    nc = bacc.Bacc("TRN2", target_bir_lowering=False, debug=False, num_devices=NCORES)
    # 11 blocks ride 110 partitions (10 features x 11 blocks); cols 0:110 hold
    # the block-diagonal T, cols 112+g*128 hold block-group g's h.T grid, so
    # each matmul covers 11 batch blocks at out-free 110.
    hp = nc.dram_tensor("hp", [110, 112 + 6 * 128], F16, kind="ExternalInput")
    ob = nc.dram_tensor("ob", [128, NBLK * 10], F16, kind="ExternalOutput")

    hp_sb = nc.alloc_sbuf_tensor("hp_sb", [110, 112 + 6 * 128], F16)
    ob_sb = nc.alloc_sbuf_tensor("ob_sb", [128, NBLK * 10], F16)
    ps1 = nc.alloc_psum_tensor("ps1", [128, 330], F32)
    ps2 = nc.alloc_psum_tensor("ps2", [128, 310], F32)

    # burn the ID range stage1 uses so stale (deliberately unwaited) output
    # sems from either stage can never alias the other's across executions
    for i in range(len(CHUNKS) + 4):
        nc.alloc_semaphore(f"pad{i}")
    s_hp = nc.alloc_semaphore("s_hp")
    s_hp2 = nc.alloc_semaphore("s_hp2")
    s_mm = nc.alloc_semaphore("s_mm")
    s_cp = nc.alloc_semaphore("s_cp")
    s_ob = nc.alloc_semaphore("s_ob")

    nc.sync.dma_start(hp_sb[:, 0:752], hp[:, 0:752]).then_inc(s_hp, 16)
    nc.sync.dma_start(hp_sb[:, 752:880], hp[:, 752:880]).then_inc(s_hp2, 16)

    nc.tensor.wait_ge(s_hp, 16)
    b0 = 0
    for g, nb in enumerate(G2):
        ps, po = (ps1, 0) if g < 3 else (ps2, 330)
        col = 112 + g * 128
        if g == 5:
            nc.tensor.wait_ge(s_hp2, 16)
        mm = nc.tensor.matmul(
            ps[:, b0 * 10 - po:(b0 + nb) * 10 - po],
            hp_sb[0:10 * nb, col:col + 128],
            hp_sb[0:10 * nb, 0:nb * 10],
            start=True,
            stop=True,
        )
        if g in (2, 5):
            mm.then_inc(s_mm)
        b0 += nb

    nc.scalar.wait_ge(s_mm, 1)
    nc.scalar.activation(
        ob_sb[:, 0:330], ps1[:], mybir.ActivationFunctionType.Copy
    ).then_inc(s_cp)
    nc.vector.wait_ge(s_mm, 2)
    nc.vector.tensor_copy(ob_sb[:, 330:640], ps2[:]).then_inc(s_cp)

    nc.sync.wait_ge(s_cp, 2)
    nc.sync.dma_start(ob[:], ob_sb[:]).then_inc(s_ob, 16)

    nc.gpsimd.wait_ge(s_cp, 2)
    for s in (s_hp, s_hp2, s_mm, s_cp):
        nc.gpsimd.sem_clear(s)
    nc.finalize()
    return nc


def _chain_host(s1, S, W0, b0, g0, beta0, Ws, bs, gs, betas, Wf, bf):
    """Collapse BN chain on global moments of h = x@W0.T (no bias). float64.
    Returns Tmat [10,10], r [10] with out = h @ Tmat + r."""
    m = s1.astype(np.float64) / B
    C = S.astype(np.float64) / B - np.outer(m, m)
    g0 = g0.astype(np.float64)
    var0 = np.diag(C).copy()
    A = np.diag(g0 / np.sqrt(var0 + EPS))
    d = beta0.astype(np.float64).copy()
    Ws64 = Ws.astype(np.float64)
    gs64 = gs.astype(np.float64)
    betas64 = betas.astype(np.float64)
    for k in range(Ws64.shape[0]):
        Ak = A @ Ws64[k].T
        var = np.einsum("ij,ik,kj->j", Ak, C, Ak)
        A = Ak * (gs64[k] / np.sqrt(var + EPS))[None, :]
        d = betas64[k].copy()
    Tmat = A @ Wf.astype(np.float64).T
    r = d @ Wf.astype(np.float64).T + bf.astype(np.float64)
    # fold bias b0 and centering: out = (h + b0 - (m + b0)) @ Tmat + r
    return Tmat, (r - m @ Tmat)


def _blocks_to_rows(a):
    """[128, NBLK*10] device layout -> [BC, 10]."""
    return np.ascontiguousarray(
        a.reshape(128, NBLK, 10).transpose(1, 0, 2).reshape(BC, 10)
    )


def kernel(**inputs):
    from concourse.bass_utils import run_bass_kernel_spmd
    import ml_dtypes

    E3 = ml_dtypes.float8_e3m4
    inputs = {k: np.asarray(v, dtype=np.float32) for k, v in inputs.items()}
    x = inputs["x"]
    W0 = inputs["W0"]

    if "nc1" not in _cache:
        _cache["nc1"] = _build_stage1()
    if "nc2" not in _cache:
        _cache["nc2"] = _build_stage2()

    # ---- host marshalling for stage 1 ----
    x8t = x.astype(E3).T                             # [784, B] view; d = k*112+p
    wm = np.empty((KP, KC * 10), dtype=np.float16)
    for k in range(KC):
        wm[:, k * 10:(k + 1) * 10] = W0[:, k * KP:(k + 1) * KP].T
    coff = np.concatenate([[0], np.cumsum(CHUNKS)])
    in1 = []
    for c in range(NCORES):
        xc3 = np.ascontiguousarray(x8t[:, c * BC:(c + 1) * BC]).reshape(KC, KP, BC)
        blob = np.empty(KC * KP * BC, dtype=E3)
        for i, W in enumerate(CHUNKS):
            pos = KP * KC * coff[i]
            blob[pos:pos + KP * KC * W] = (
                xc3[:, :, coff[i]:coff[i + 1]].transpose(1, 0, 2).ravel()
            )
        in1.append({"xp": blob, "wm": wm})
    res1 = run_bass_kernel_spmd(_cache["nc1"], in1, core_ids=list(range(NCORES)))

    # ---- host: moments of the computed fp16 h, then the tiny BN chain ----
    h16 = np.concatenate(
        [_blocks_to_rows(np.asarray(res1.results[c]["hb"])) for c in range(NCORES)],
        axis=0,
    )                                                # [B, 10] fp16
    h64 = h16.astype(np.float64)
    s1 = h64.sum(axis=0)
    S = h64.T @ h64
    Tmat, r = _chain_host(
        s1, S,
        W0, inputs["b0"], inputs["g0"], inputs["beta0"],
        inputs["Ws"], inputs["bs"], inputs["gs"], inputs["betas"],
        inputs["Wf"], inputs["bf"],
    )

    # ---- stage 2: out = h @ T on device, 11 blocks per matmul; r on host ----
    tm16 = Tmat.astype(np.float16)                   # [10, 10]
    in2 = []
    for c in range(NCORES):
        h3 = h16[c * BC:(c + 1) * BC].reshape(NBLK, 128, 10)
        hpc = np.zeros((110, 112 + 6 * 128), dtype=np.float16)
        for j in range(11):
            hpc[j * 10:(j + 1) * 10, j * 10:(j + 1) * 10] = tm16
        b0 = 0
        for g, nb in enumerate(G2):
            grid = h3[b0:b0 + nb].transpose(0, 2, 1).reshape(nb * 10, 128)
            col = 112 + g * 128
            hpc[0:nb * 10, col:col + 128] = grid
            b0 += nb
        in2.append({"hp": hpc})
    res2 = run_bass_kernel_spmd(_cache["nc2"], in2, core_ids=list(range(NCORES)))

    out = np.concatenate(
        [_blocks_to_rows(np.asarray(res2.results[c]["ob"])) for c in range(NCORES)],
        axis=0,
    )
    return out.astype(np.float32) + r.astype(np.float32)[None, :]


# revision 5
# speedup vs baseline: 1.0007x; 1.0007x over previous
"""Trainium2 Bass kernel for nn_DeepLinear (784->10 linear + BN, 62x(10->10 linear + BN), 10->10 linear).

Math: training-mode BN fixes each layer's output batch mean (beta) and variance
(gamma^2), so every layer past the first acts *linearly* on the centered
activations of h = x @ W0.T. The whole net collapses to:
    h  = x @ W0.T                      (heavy; on device, data-parallel over batch)
    mu = mean(h), C = cov(h)           (global batch moments; host, float64)
    T, r = 62-layer chain of 10x10 covariance algebra   (tiny; host, float64)
    out = h @ T + r                    (light; on device)

Stage 1 streams x as fp8e3m4 (1 byte/elem; ~1.8% rms quantization noise per
element -> ~1.4e-2 relative output error, inside the 2e-2 gate) and runs the
matmul with each x block as the PE *stationary* operand ([112,128]) and the
fp16 weights as the 10-wide moving operand, so a 128-row batch block costs ~10
PE rows and the result lands directly in [batch, feature] layout. h returns as
fp16; the host computes the batch moments from those same fp16 values (the
sync-BN all-reduce), collapses the BN chain, and launches stage 2, which
applies T to 11 batch blocks per matmul via a block-diagonal layout (10
features x 11 blocks on 110 partitions). The bias r is added on host.

Both launches are raw bass (no TileContext): hand-rolled semaphores, per-chunk
DMA-completion sems (DMA engines complete out of order), x-chunk loads on SP
with weight loads and the mid output write on ACT so the serialized DMA stream
stays gap-free, and a Pool epilogue that re-zeros semaphores so repeat
executions of the same NEFF stay correct. Stage 2 burns stage 1's semaphore ID
range so stale (deliberately unwaited) output sems can never alias.
"""

import numpy as np

EPS = 1e-5
B = 65536
D = 784
NCORES = 8
BC = B // NCORES          # 8192 rows per core
KP = 112                  # contraction partitions per chunk (7 * 112 = 784)
KC = 7                    # contraction chunks
BLK = 128                 # batch rows per matmul block
NBLK = BC // BLK          # 64 blocks per core

CHUNKS = [512] * 15 + [384, 128]            # batch cols per x DMA chunk
GROUPS = [(0, 4), (4, 4), (8, 4), (12, 4), (16, 1)]   # (first chunk, n chunks)
G2 = [11, 11, 11, 11, 11, 9]                # stage-2 blocks per matmul

_cache = {}


def _build_stage1():
    import concourse.bacc as bacc
    import concourse.mybir as mybir

    F16 = mybir.dt.float16
    F32 = mybir.dt.float32
    E3 = mybir.dt.float8e3

    nc = bacc.Bacc("TRN2", target_bir_lowering=False, debug=False, num_devices=NCORES)
    xp = nc.dram_tensor("xp", [KC * KP * BC], E3, kind="ExternalInput")
    wm = nc.dram_tensor("wm", [KP, KC * 10], F16, kind="ExternalInput")
    hb = nc.dram_tensor("hb", [128, NBLK * 10], F16, kind="ExternalOutput")

    coff = np.concatenate([[0], np.cumsum(CHUNKS)])
    NCH = len(CHUNKS)
    XBUF = 4

    wm_sb = nc.alloc_sbuf_tensor("wm_sb", [KP, KC * 10], F16)
    hn_sb = nc.alloc_sbuf_tensor("hn_sb", [128, NBLK * 10], F16)
    xts = [nc.alloc_sbuf_tensor(f"xt{i}", [KP, KC, 512], E3) for i in range(XBUF)]
    xt_tail = {
        W: nc.alloc_sbuf_tensor(f"xt_{W}", [KP, KC, W], E3)
        for W in sorted(set(CHUNKS) - {512})
    }
    pss = [
        nc.alloc_psum_tensor(
            f"ps{g}", [128, (coff[c0 + n] - coff[c0]) // BLK * 10], F32
        )
        for g, (c0, n) in enumerate(GROUPS)
    ]

    s_w = nc.alloc_semaphore("s_w")                              # wm loaded
    s_xc = [nc.alloc_semaphore(f"s_x{c}") for c in range(NCH)]   # per-chunk
    s_rd = nc.alloc_semaphore("s_rd")    # chunks fully consumed by PE
    s_cp = nc.alloc_semaphore("s_cp")    # group copies done
    s_ob = nc.alloc_semaphore("s_ob")    # output writes (unwaited)

    nc.scalar.dma_start(wm_sb[:], wm[:]).then_inc(s_w, 16)

    # all x-chunk loads on SP; ring of XBUF buffers for the 512-col chunks
    ring = 0
    for c in range(NCH):
        W = CHUNKS[c]
        if W == 512:
            xt = xts[ring % XBUF]
            if ring >= XBUF:
                nc.sync.wait_ge(s_rd, ring - XBUF + 1)
            ring += 1
        else:
            xt = xt_tail[W]
        pos = KP * KC * coff[c]
        nc.sync.dma_start(
            xt[:, :, 0:W],
            xp[pos:pos + KP * KC * W].rearrange("(p k w) -> p k w", p=KP, k=KC),
        ).then_inc(s_xc[c], 16)

    nc.tensor.wait_ge(s_w, 16)
    ring = 0
    for g, (c0, nch) in enumerate(GROUPS):
        b0 = coff[c0] // BLK
        ps = pss[g]
        for c in range(c0, c0 + nch):
            W = CHUNKS[c]
            if W == 512:
                xt = xts[ring % XBUF]
                ring += 1
            else:
                xt = xt_tail[W]
            nc.tensor.wait_ge(s_xc[c], 16)
            mm = None
            for b in range(W // BLK):
                blk = (coff[c] // BLK - b0) + b
                for k in range(KC):
                    mm = nc.tensor.matmul(
                        ps[:, blk * 10:(blk + 1) * 10],
                        xt[:, k, b * BLK:(b + 1) * BLK],
                        wm_sb[:, k * 10:(k + 1) * 10],
                        start=(k == 0),
                        stop=(k == KC - 1),
                    )
            mm.then_inc(s_rd)
        nb = (coff[c0 + nch] - coff[c0]) // BLK
        if g == len(GROUPS) - 1:
            # last (1-block) group copies on ACT so it runs parallel to g3's
            # DVE copy in the tail
            nc.scalar.wait_ge(s_rd, c0 + nch)
            nc.scalar.activation(
                hn_sb[:, b0 * 10:(b0 + nb) * 10], ps[:],
                mybir.ActivationFunctionType.Copy,
            ).then_inc(s_cp)
        else:
            nc.vector.wait_ge(s_rd, c0 + nch)
            nc.vector.tensor_copy(
                hn_sb[:, b0 * 10:(b0 + nb) * 10], ps[:]
            ).then_inc(s_cp)

    # writes: [0:320] mid-stream on ACT after groups 0-1; rest on SP at the end
    nc.scalar.wait_ge(s_cp, 3)
    nc.scalar.dma_start(hb[:, 0:320], hn_sb[:, 0:320]).then_inc(s_ob, 16)
    nc.sync.wait_ge(s_cp, len(GROUPS))
    nc.sync.dma_start(hb[:, 320:640], hn_sb[:, 320:640]).then_inc(s_ob, 16)

    # epilogue: re-zero semaphores for repeat executions (off critical path)
    nc.gpsimd.wait_ge(s_cp, len(GROUPS))
    for s in (s_w, s_rd, s_cp, *s_xc):
        nc.gpsimd.sem_clear(s)
    nc.finalize()
    return nc


def _build_stage2():
    import concourse.bacc as bacc
    import concourse.mybir as mybir

    F16 = mybir.dt.float16
    F32 = mybir.dt.float32

    nc = bacc.Bacc("TRN2", target_bir_lowering=False, debug=False, num_devices=NCORES)
    # 11 blocks ride 110 partitions (10 features x 11 blocks); cols 0:110 hold
    # the block-diagonal T, cols 112+g*128 hold block-group g's h.T grid, so
    # each matmul covers 11 batch blocks at out-free 110.
    hp = nc.dram_tensor("hp", [110, 112 + 6 * 128], F16, kind="ExternalInput")
    ob = nc.dram_tensor("ob", [128, NBLK * 10], F16, kind="ExternalOutput")

    hp_sb = nc.alloc_sbuf_tensor("hp_sb", [110, 112 + 6 * 128], F16)
    ob_sb = nc.alloc_sbuf_tensor("ob_sb", [128, NBLK * 10], F16)
    ps1 = nc.alloc_psum_tensor("ps1", [128, 330], F32)
    ps2 = nc.alloc_psum_tensor("ps2", [128, 310], F32)

    # burn the ID range stage1 uses so stale (deliberately unwaited) output
    # sems from either stage can never alias the other's across executions
    for i in range(len(CHUNKS) + 4):
        nc.alloc_semaphore(f"pad{i}")
    s_hp = nc.alloc_semaphore("s_hp")
    s_hp2 = nc.alloc_semaphore("s_hp2")
    s_mm = nc.alloc_semaphore("s_mm")
    s_cp = nc.alloc_semaphore("s_cp")
    s_ob = nc.alloc_semaphore("s_ob")

    nc.sync.dma_start(hp_sb[:, 0:624], hp[:, 0:624]).then_inc(s_hp, 16)
    nc.sync.dma_start(hp_sb[:, 624:880], hp[:, 624:880]).then_inc(s_hp2, 16)

    nc.tensor.wait_ge(s_hp, 16)
    b0 = 0
    for g, nb in enumerate(G2):
        ps, po = (ps1, 0) if g < 3 else (ps2, 330)
        col = 112 + g * 128
        if g == 4:
            nc.tensor.wait_ge(s_hp2, 16)
        mm = nc.tensor.matmul(
            ps[:, b0 * 10 - po:(b0 + nb) * 10 - po],
            hp_sb[0:10 * nb, col:col + 128],
            hp_sb[0:10 * nb, 0:nb * 10],
            start=True,
            stop=True,
        )
        if g in (2, 5):
            mm.then_inc(s_mm)
        b0 += nb

    nc.scalar.wait_ge(s_mm, 1)
    nc.scalar.activation(
        ob_sb[:, 0:330], ps1[:], mybir.ActivationFunctionType.Copy
    ).then_inc(s_cp)
    nc.vector.wait_ge(s_mm, 2)
    nc.vector.tensor_copy(ob_sb[:, 330:640], ps2[:]).then_inc(s_cp)

    nc.sync.wait_ge(s_cp, 2)
    nc.sync.dma_start(ob[:], ob_sb[:]).then_inc(s_ob, 16)

    nc.gpsimd.wait_ge(s_cp, 2)
    for s in (s_hp, s_hp2, s_mm, s_cp):
        nc.gpsimd.sem_clear(s)
    nc.finalize()
    return nc


def _chain_host(s1, S, W0, b0, g0, beta0, Ws, bs, gs, betas, Wf, bf):
    """Collapse BN chain on global moments of h = x@W0.T (no bias). float64.
    Returns Tmat [10,10], r [10] with out = h @ Tmat + r."""
    m = s1.astype(np.float64) / B
    C = S.astype(np.float64) / B - np.outer(m, m)
    g0 = g0.astype(np.float64)
    var0 = np.diag(C).copy()
    A = np.diag(g0 / np.sqrt(var0 + EPS))
    d = beta0.astype(np.float64).copy()
    Ws64 = Ws.astype(np.float64)
    gs64 = gs.astype(np.float64)
    betas64 = betas.astype(np.float64)
    for k in range(Ws64.shape[0]):
        Ak = A @ Ws64[k].T
        var = np.einsum("ij,ik,kj->j", Ak, C, Ak)
        A = Ak * (gs64[k] / np.sqrt(var + EPS))[None, :]
        d = betas64[k].copy()
    Tmat = A @ Wf.astype(np.float64).T
    r = d @ Wf.astype(np.float64).T + bf.astype(np.float64)
    # fold bias b0 and centering: out = (h + b0 - (m + b0)) @ Tmat + r
    return Tmat, (r - m @ Tmat)


def _blocks_to_rows(a):
    """[128, NBLK*10] device layout -> [BC, 10]."""
    return np.ascontiguousarray(
        a.reshape(128, NBLK, 10).transpose(1, 0, 2).reshape(BC, 10)
    )


def kernel(**inputs):
    from concourse.bass_utils import run_bass_kernel_spmd
    import ml_dtypes

    E3 = ml_dtypes.float8_e3m4
    inputs = {k: np.asarray(v, dtype=np.float32) for k, v in inputs.items()}
    x = inputs["x"]
    W0 = inputs["W0"]

    if "nc1" not in _cache:
        _cache["nc1"] = _build_stage1()
    if "nc2" not in _cache:
        _cache["nc2"] = _build_stage2()

    # ---- host marshalling for stage 1 ----
    x8t = x.astype(E3).T                             # [784, B] view; d = k*112+p
    wm = np.empty((KP, KC * 10), dtype=np.float16)
    for k in range(KC):
        wm[:, k * 10:(k + 1) * 10] = W0[:, k * KP:(k + 1) * KP].T
    coff = np.concatenate([[0], np.cumsum(CHUNKS)])
    in1 = []
    for c in range(NCORES):
        xc3 = np.ascontiguousarray(x8t[:, c * BC:(c + 1) * BC]).reshape(KC, KP, BC)
        blob = np.empty(KC * KP * BC, dtype=E3)
        for i, W in enumerate(CHUNKS):
            pos = KP * KC * coff[i]
            blob[pos:pos + KP * KC * W] = (
                xc3[:, :, coff[i]:coff[i + 1]].transpose(1, 0, 2).ravel()
            )
        in1.append({"xp": blob, "wm": wm})
    res1 = run_bass_kernel_spmd(_cache["nc1"], in1, core_ids=list(range(NCORES)))

    # ---- host: moments of the computed fp16 h, then the tiny BN chain ----
    h16 = np.concatenate(
        [_blocks_to_rows(np.asarray(res1.results[c]["hb"])) for c in range(NCORES)],
        axis=0,
    )                                                # [B, 10] fp16
    h64 = h16.astype(np.float64)
    s1 = h64.sum(axis=0)
    S = h64.T @ h64
    Tmat, r = _chain_host(
        s1, S,
        W0, inputs["b0"], inputs["g0"], inputs["beta0"],
        inputs["Ws"], inputs["bs"], inputs["gs"], inputs["betas"],
        inputs["Wf"], inputs["bf"],
    )

    # ---- stage 2: out = h @ T on device, 11 blocks per matmul; r on host ----
    tm16 = Tmat.astype(np.float16)                   # [10, 10]
    in2 = []
    for c in range(NCORES):
        h3 = h16[c * BC:(c + 1) * BC].reshape(NBLK, 128, 10)
        hpc = np.zeros((110, 112 + 6 * 128), dtype=np.float16)
        for j in range(11):
            hpc[j * 10:(j + 1) * 10, j * 10:(j + 1) * 10] = tm16
        b0 = 0
        for g, nb in enumerate(G2):
            grid = h3[b0:b0 + nb].transpose(0, 2, 1).reshape(nb * 10, 128)
            col = 112 + g * 128
            hpc[0:nb * 10, col:col + 128] = grid
            b0 += nb
        in2.append({"hp": hpc})
    res2 = run_bass_kernel_spmd(_cache["nc2"], in2, core_ids=list(range(NCORES)))

    out = np.concatenate(
        [_blocks_to_rows(np.asarray(res2.results[c]["ob"])) for c in range(NCORES)],
        axis=0,
    )
    return out.astype(np.float32) + r.astype(np.float32)[None, :]
